# revision 5
# baseline (speedup 1.0000x reference)
"""Multi-head self-attention (RoPE, causal) Trainium2 Bass kernel.

Sharding: 8 cores = 4 batches x 2 head-groups (8 heads each).
Per core the device kernel computes, for its batch b and head-group g:
    q/k/v = x_b @ W*[:, g] (+bias), RoPE on q/k, causal softmax attention,
    partial out-projection y @ Wo[g]  -> [2048, 1024] (f32).
Host sums the two head-group partials per batch and adds bo.

Device layouts (per core):
    xT   [1024, 2048] bf16   x_b transposed (host-prepped: sharding step)
    qT'/kT' [128, 4, 2048]   projected+roped, dims-on-partitions
    v    [128, 16kb, 8h, 65] bf16, col 64 = ones (softmax denominator trick)
    att  [128, 17408] bf16   exp(scores^T) per head, causal-trapezoid packed
    y    [128, 16qb, 128] x4 per-head-pair attention outputs (q-on-partitions)
    yT   [128, 4, 2048] bf16 transposed y for the out-projection
"""

import os
import sys

import numpy as np

for _p in ("/opt/trn_rl_repo", "/root/.axon_site/_ro/trn_rl_repo"):
    if os.path.isdir(_p) and _p not in sys.path:
        sys.path.append(_p)

import ml_dtypes  # noqa: E402

BF16 = ml_dtypes.bfloat16

B, S, D_MODEL = 4, 2048, 1024
N_HEADS, HEAD_DIM = 16, 64
N_CORES = 8
HG = 2                      # head groups
HPC = N_HEADS // HG         # heads per core = 8
DL = HPC * HEAD_DIM         # local dims per core = 512
SCALE = HEAD_DIM ** -0.5
P = 128
KC = D_MODEL // P           # k chunks in projections = 8
MB = DL // P                # m blocks (head pairs) = 4
NKB = S // P                # 128-row blocks of sequence = 16
QK_PSUM_W = 1536            # scores psum tile width (3 banks)

# packed causal-trapezoid offsets: att row-block ck covers q in [128*ck, S)
ATT_OFF = [0] * (NKB + 1)
for _ck in range(NKB):
    ATT_OFF[_ck + 1] = ATT_OFF[_ck] + (S - P * _ck)
ATT_TOT = ATT_OFF[NKB]      # 17408

_CACHE = {}


def _build_bass():
    import concourse.tile as tile
    from concourse import bacc, mybir

    dt = mybir.dt
    nc = bacc.Bacc("TRN2", target_bir_lowering=False, debug=False)

    def din(name, shape):
        return nc.dram_tensor(name, shape, dt.bfloat16, kind="ExternalInput").ap()

    xT_d = din("xT", [D_MODEL, S])
    wq_d = din("wq", [D_MODEL, DL])
    wk_d = din("wk", [D_MODEL, DL])
    wv_d = din("wv", [D_MODEL, DL])
    wo_d = din("wo", [DL, D_MODEL])
    bq_d = din("bq", [1, DL])
    bk_d = din("bk", [1, DL])
    bv_d = din("bv", [1, DL])
    cos_d = din("cosT", [P, S])
    sin_d = din("sinT", [P, S])
    perm_d = din("permT", [P, P])
    tri_d = din("tri", [P, P])
    ident_d = din("ident", [P, P])
    o_d = nc.dram_tensor("o", [S, D_MODEL], dt.float32, kind="ExternalOutput").ap()

    FCopy = mybir.ActivationFunctionType.Copy
    FExp = mybir.ActivationFunctionType.Exp

    with tile.TileContext(nc) as tc:
        # ---- persistent pools (live whole kernel) ----
        with (
            tc.tile_pool(name="persist", bufs=1) as persist,
            tc.tile_pool(name="small", bufs=1) as small,
        ):
            wo_sb = persist.tile([P, MB, D_MODEL], dt.bfloat16)
            nc.sync.dma_start(out=wo_sb, in_=wo_d.rearrange("(m p) n -> p m n", p=P))
            qTf = persist.tile([P, MB, S], dt.bfloat16, tag="qTf")
            kTf = persist.tile([P, MB, S], dt.bfloat16, tag="kTf")
            v_sb = persist.tile([P, NKB, HPC, HEAD_DIM + 1], dt.bfloat16, tag="v_sb")
            yT_all = persist.tile([P, MB, S], dt.bfloat16, tag="yT")
            y_mb = [persist.tile([P, NKB, P], dt.bfloat16, tag=f"y_mb{m}",
                                 name=f"y_mb{m}")
                    for m in range(MB)]

            tri_sb = small.tile([P, P], dt.bfloat16, tag="tri")
            nc.sync.dma_start(out=tri_sb, in_=tri_d)
            ident_sb = small.tile([P, P], dt.bfloat16, tag="ident")
            nc.sync.dma_start(out=ident_sb, in_=ident_d)
            ones_sb = small.tile([1, DL], dt.bfloat16, tag="ones")
            nc.vector.memset(ones_sb, 1.0)

            # ones columns of v (softmax denominator accumulators)
            nc.vector.memset(v_sb[:, :, :, HEAD_DIM:HEAD_DIM + 1], 1.0)

            # ================= Stage B: projections + RoPE =================
            with (
                tc.tile_pool(name="bweights", bufs=1) as bweights,
                tc.tile_pool(name="bstage", bufs=3) as bstage,
                tc.tile_pool(name="proj_ps", bufs=3, space="PSUM") as proj_ps,
                tc.tile_pool(name="rot_ps", bufs=2, space="PSUM") as rot_ps,
            ):
                xT_sb = bweights.tile([P, KC, S], dt.bfloat16, tag="xT")
                nc.sync.dma_start(
                    out=xT_sb, in_=xT_d.rearrange("(kc p) s -> p kc s", p=P))
                w_sbs = {}
                b_sbs = {}
                for nm, wd, bd in (("q", wq_d, bq_d), ("k", wk_d, bk_d),
                                   ("v", wv_d, bv_d)):
                    w_sbs[nm] = bweights.tile([P, KC, DL], dt.bfloat16, tag=f"w{nm}", name=f"w{nm}")
                    nc.sync.dma_start(
                        out=w_sbs[nm], in_=wd.rearrange("(kc p) n -> p kc n", p=P))
                    b_sbs[nm] = small.tile([1, DL], dt.bfloat16, tag=f"b{nm}", name=f"b{nm}")
                    nc.sync.dma_start(out=b_sbs[nm], in_=bd)
                cos_sb = bweights.tile([P, S], dt.bfloat16, tag="cos")
                nc.sync.dma_start(out=cos_sb, in_=cos_d)
                sin_sb = bweights.tile([P, S], dt.bfloat16, tag="sin")
                nc.sync.dma_start(out=sin_sb, in_=sin_d)
                perm_sb = small.tile([P, P], dt.bfloat16, tag="perm")
                nc.sync.dma_start(out=perm_sb, in_=perm_d)

                NT = S // DL  # 4 tiles of 512 along sequence
                for nm, dest in (("q", qTf), ("k", kTf)):
                    w_sb, b_sb = w_sbs[nm], b_sbs[nm]
                    for m in range(MB):
                        for t in range(NT):
                            ps = proj_ps.tile([P, DL], dt.float32, tag="proj")
                            for kc in range(KC):
                                nc.tensor.matmul(
                                    ps, lhsT=w_sb[:, kc, m * P:(m + 1) * P],
                                    rhs=xT_sb[:, kc, t * DL:(t + 1) * DL],
                                    start=(kc == 0), stop=False)
                            nc.tensor.matmul(  # + bias ⊗ ones
                                ps, lhsT=b_sb[:, m * P:(m + 1) * P],
                                rhs=ones_sb, start=False, stop=True)
                            raw = bstage.tile([P, DL], dt.bfloat16, tag="raw")
                            nc.vector.tensor_copy(out=raw, in_=ps)
                            rps = rot_ps.tile([P, DL], dt.float32, tag="rot")
                            nc.tensor.matmul(rps, lhsT=perm_sb, rhs=raw,
                                             start=True, stop=True)
                            rot = bstage.tile([P, DL], dt.bfloat16, tag="rot_sb")
                            nc.vector.tensor_copy(out=rot, in_=rps)
                            t1 = bstage.tile([P, DL], dt.bfloat16, tag="t1")
                            nc.vector.tensor_mul(
                                t1, raw, cos_sb[:, t * DL:(t + 1) * DL])
                            t2 = bstage.tile([P, DL], dt.bfloat16, tag="t2")
                            nc.vector.tensor_mul(
                                t2, rot, sin_sb[:, t * DL:(t + 1) * DL])
                            nc.vector.tensor_add(
                                dest[:, m, t * DL:(t + 1) * DL], t1, t2)

                # v projection: natural [seq, dims] layout + ones cols
                for kb in range(NKB):
                    ps = proj_ps.tile([P, DL], dt.float32, tag="proj")
                    for kc in range(KC):
                        nc.tensor.matmul(
                            ps, lhsT=xT_sb[:, kc, kb * P:(kb + 1) * P],
                            rhs=w_sbs["v"][:, kc, :],
                            start=(kc == 0), stop=False)
                    nc.tensor.matmul(
                        ps, lhsT=ones_sb[:, :P], rhs=b_sbs["v"],
                        start=False, stop=True)
                    nc.vector.tensor_copy(
                        out=v_sb[:, kb, :, 0:HEAD_DIM],
                        in_=ps.rearrange("p (h d) -> p h d", h=HPC))

            # ================= Stage C: attention per head =================
            with (
                tc.tile_pool(name="att_pool", bufs=2) as att_pool,
                tc.tile_pool(name="rtile", bufs=4) as rtile,
                tc.tile_pool(name="qk_ps", bufs=2, space="PSUM") as qk_ps,
                tc.tile_pool(name="y_ps", bufs=1, space="PSUM") as y_ps_pool,
            ):
                for h in range(HPC):
                    m, po = h // 2, (h % 2) * HEAD_DIM
                    qh = qTf[po:po + HEAD_DIM, m, :]
                    kh = kTf[po:po + HEAD_DIM, m, :]
                    att = att_pool.tile([P, ATT_TOT], dt.bfloat16, tag="att")

                    # C1: scores^T = k_ck^T q (per 128-key block), exp, mask
                    for ck in range(NKB):
                        w = S - ck * P
                        base = ck * P
                        off = ATT_OFF[ck]
                        for s0 in range(0, w, QK_PSUM_W):
                            sw = min(QK_PSUM_W, w - s0)
                            ps = qk_ps.tile([P, QK_PSUM_W], dt.float32, tag="qk")
                            for u0 in range(0, sw, DL):
                                uw = min(DL, sw - u0)
                                nc.tensor.matmul(
                                    ps[:, u0:u0 + uw],
                                    lhsT=kh[:, ck * P:(ck + 1) * P],
                                    rhs=qh[:, base + s0 + u0:base + s0 + u0 + uw],
                                    start=True, stop=True)
                            nc.scalar.activation(
                                out=att[:, off + s0:off + s0 + sw],
                                in_=ps[:, 0:sw], func=FExp, scale=SCALE)
                        # causal mask on the diagonal 128x128 block
                        nc.vector.tensor_mul(
                            att[:, off:off + P], att[:, off:off + P], tri_sb)

                    # C2: y[qb] = sum_ck att[ck,qb-block]^T @ v_aug[ck]
                    for qb in range(NKB):
                        yp = y_ps_pool.tile([P, 2, HEAD_DIM + 1], dt.float32,
                                            tag="y")
                        ys = yp[:, qb % 2, :]
                        for ck in range(qb + 1):
                            a0 = ATT_OFF[ck] + (qb - ck) * P
                            nc.tensor.matmul(
                                ys, lhsT=att[:, a0:a0 + P],
                                rhs=v_sb[:, ck, h, :],
                                start=(ck == 0), stop=(ck == qb))
                        r = rtile.tile([P, 1], dt.float32, tag="r")
                        nc.vector.reciprocal(r, ys[:, HEAD_DIM:HEAD_DIM + 1])
                        nc.vector.tensor_scalar(
                            y_mb[m][:, qb, po:po + HEAD_DIM],
                            ys[:, 0:HEAD_DIM], r, None,
                            mybir.AluOpType.mult)

            # ============ Stage D: transpose y, out-projection ============
            with (
                tc.tile_pool(name="osb", bufs=3) as osb,
                tc.tile_pool(name="tp_ps", bufs=2, space="PSUM") as tp_ps,
                tc.tile_pool(name="o_ps", bufs=3, space="PSUM") as o_ps_pool,
            ):
                for m in range(MB):
                    for q4 in range(NKB // 4):
                        tp = tp_ps.tile([P, 4 * P], dt.bfloat16, tag="tp")
                        for j in range(4):
                            nc.tensor.transpose(
                                tp[:, j * P:(j + 1) * P],
                                y_mb[m][:, q4 * 4 + j, :], ident_sb)
                        nc.vector.tensor_copy(
                            out=yT_all[:, m, q4 * 4 * P:(q4 + 1) * 4 * P], in_=tp)

                for qb in range(NKB):
                    for t in range(2):
                        ps = o_ps_pool.tile([P, DL], dt.float32, tag="o")
                        for m in range(MB):
                            nc.tensor.matmul(
                                ps, lhsT=yT_all[:, m, qb * P:(qb + 1) * P],
                                rhs=wo_sb[:, m, t * DL:(t + 1) * DL],
                                start=(m == 0), stop=(m == MB - 1))
                        ob = osb.tile([P, DL], dt.float32, tag="ob")
                        nc.scalar.activation(out=ob, in_=ps, func=FCopy)
                        nc.sync.dma_start(
                            out=o_d[qb * P:(qb + 1) * P, t * DL:(t + 1) * DL],
                            in_=ob)

    nc.compile()
    return nc


def _host_tables():
    pos = np.arange(S, dtype=np.float32)
    freq = np.arange(0, HEAD_DIM, 2, dtype=np.float32) / HEAD_DIM
    inv_freq = 1.0 / (10000.0 ** freq)
    angles = np.outer(pos, inv_freq)                    # [S, 32]
    angles = np.repeat(angles, 2, axis=-1)              # [S, 64]
    cosT = np.tile(np.cos(angles).T, (2, 1))            # [128, S]
    sinT = np.tile(np.sin(angles).T, (2, 1))
    # rot(q)[2i] = -q[2i+1]; rot(q)[2i+1] = q[2i]; permT[j, p] = coeff of q[j]
    p64 = np.zeros((HEAD_DIM, HEAD_DIM), np.float32)
    for i in range(HEAD_DIM // 2):
        p64[2 * i + 1, 2 * i] = -1.0
        p64[2 * i, 2 * i + 1] = 1.0
    permT = np.zeros((P, P), np.float32)
    permT[:HEAD_DIM, :HEAD_DIM] = p64
    permT[HEAD_DIM:, HEAD_DIM:] = p64
    tri = np.triu(np.ones((P, P), np.float32))          # keep k<=q in [k,q]
    ident = np.eye(P, dtype=np.float32)
    return (cosT.astype(BF16), sinT.astype(BF16), permT.astype(BF16),
            tri.astype(BF16), ident.astype(BF16))


def kernel(x, Wq, bq, Wk, bk, Wv, bv, Wo, bo):
    from concourse.bass_utils import run_bass_kernel_spmd

    x = np.asarray(x, np.float32)
    Wq, Wk, Wv, Wo = (np.asarray(a, np.float32) for a in (Wq, Wk, Wv, Wo))
    bq, bk, bv, bo = (np.asarray(a, np.float32) for a in (bq, bk, bv, bo))

    if "nc" not in _CACHE:
        _CACHE["nc"] = _build_bass()
    nc = _CACHE["nc"]

    cosT, sinT, permT, tri, ident = _host_tables()
    consts = {"cosT": cosT, "sinT": sinT, "permT": permT, "tri": tri,
              "ident": ident}

    xTs = [np.ascontiguousarray(x[b].T).astype(BF16) for b in range(B)]
    in_maps = []
    for c in range(N_CORES):
        b, g = c // HG, c % HG
        sl = slice(g * DL, (g + 1) * DL)
        in_maps.append({
            "xT": xTs[b],
            "wq": np.ascontiguousarray(Wq[:, sl]).astype(BF16),
            "wk": np.ascontiguousarray(Wk[:, sl]).astype(BF16),
            "wv": np.ascontiguousarray(Wv[:, sl]).astype(BF16),
            "wo": np.ascontiguousarray(Wo[sl, :]).astype(BF16),
            "bq": bq[sl].reshape(1, DL).astype(BF16),
            "bk": bk[sl].reshape(1, DL).astype(BF16),
            "bv": bv[sl].reshape(1, DL).astype(BF16),
            **consts,
        })

    res = run_bass_kernel_spmd(nc, in_maps, core_ids=list(range(N_CORES)))
    _CACHE["last_result"] = res
    out = np.empty((B, S, D_MODEL), np.float32)
    for b in range(B):
        out[b] = res.results[HG * b]["o"] + res.results[HG * b + 1]["o"]
    out += bo.astype(np.float32)
    return out


# revision 10
# speedup vs baseline: 1.0576x; 1.0576x over previous
"""Multi-head self-attention (RoPE, causal) Trainium2 Bass kernel.

Sharding: 8 cores = 4 batches x 2 head-groups (8 heads each).
Per core the device kernel computes, for its batch b and head-group g:
    q/k/v = x_b @ W*[:, g] (+bias), RoPE on q/k, causal softmax attention,
    partial out-projection y @ Wo[g]  -> [2048, 1024] (f32).
Host sums the two head-group partials per batch and adds bo.

Device layouts (per core):
    xT   [1024, 2048] bf16   x_b transposed (host-prepped: sharding step)
    qT'/kT' [128, 4, 2048]   projected+roped, dims-on-partitions
    v    [128, 16kb, 8h, 65] bf16, col 64 = ones (softmax denominator trick)
    att  [128, 17408] bf16   exp(scores^T) per head, causal-trapezoid packed
    y    [128, 16qb, 128] x4 per-head-pair attention outputs (q-on-partitions)
    yT   [128, 4, 2048] bf16 transposed y for the out-projection
"""

import os
import sys

import numpy as np

for _p in ("/opt/trn_rl_repo", "/root/.axon_site/_ro/trn_rl_repo"):
    if os.path.isdir(_p) and _p not in sys.path:
        sys.path.append(_p)

import ml_dtypes  # noqa: E402

BF16 = ml_dtypes.bfloat16

B, S, D_MODEL = 4, 2048, 1024
N_HEADS, HEAD_DIM = 16, 64
N_CORES = 8
HG = 2                      # head groups
HPC = N_HEADS // HG         # heads per core = 8
DL = HPC * HEAD_DIM         # local dims per core = 512
SCALE = HEAD_DIM ** -0.5
P = 128
KC = D_MODEL // P           # k chunks in projections = 8
MB = DL // P                # m blocks (head pairs) = 4
NKB = S // P                # 128-row blocks of sequence = 16
QK_PSUM_W = 1536            # scores psum tile width (3 banks)

# packed causal-trapezoid offsets: att row-block ck covers q in [128*ck, S)
ATT_OFF = [0] * (NKB + 1)
for _ck in range(NKB):
    ATT_OFF[_ck + 1] = ATT_OFF[_ck] + (S - P * _ck)
ATT_TOT = ATT_OFF[NKB]      # 17408

_CACHE = {}
STAGE_OF = {}


def _tag(inst, stage):
    try:
        STAGE_OF[str(inst.ins.name)] = stage
    except Exception:
        pass


def _build_bass():
    import concourse.tile as tile
    from concourse import bacc, mybir

    dt = mybir.dt
    nc = bacc.Bacc("TRN2", target_bir_lowering=False, debug=False)

    def din(name, shape):
        return nc.dram_tensor(name, shape, dt.bfloat16, kind="ExternalInput").ap()

    xT_d = din("xT", [D_MODEL, S])
    wq_d = din("wq", [D_MODEL, DL])
    wk_d = din("wk", [D_MODEL, DL])
    wv_d = din("wv", [D_MODEL, DL])
    wo_d = din("wo", [DL, D_MODEL])
    bq_d = nc.dram_tensor("bqT", [P, MB], dt.float32, kind="ExternalInput").ap()
    bk_d = nc.dram_tensor("bkT", [P, MB], dt.float32, kind="ExternalInput").ap()
    bv_d = din("bv", [1, DL])
    cos_d = din("cosT", [P, S])
    sin_d = din("sinT", [P, S])
    perm_d = din("permT", [P, P])
    tri_d = din("tri", [P, P])
    ident_d = din("ident", [P, P])
    o_d = nc.dram_tensor("o", [S, D_MODEL], dt.float32, kind="ExternalOutput").ap()

    FCopy = mybir.ActivationFunctionType.Copy
    FExp = mybir.ActivationFunctionType.Exp

    with tile.TileContext(nc) as tc:
        # ---- persistent pools (live whole kernel) ----
        with (
            tc.tile_pool(name="persist", bufs=1) as persist,
            tc.tile_pool(name="small", bufs=1) as small,
        ):
            wo_sb = persist.tile([P, MB, D_MODEL], dt.bfloat16)
            nc.sync.dma_start(out=wo_sb, in_=wo_d.rearrange("(m p) n -> p m n", p=P))
            qTf = persist.tile([P, MB, S], dt.bfloat16, tag="qTf")
            kTf = persist.tile([P, HPC, S], dt.bfloat16, tag="kTf")
            nc.vector.memset(kTf, 0.0)
            v_sb = persist.tile([P, NKB, HPC, HEAD_DIM + 1], dt.bfloat16, tag="v_sb")
            yT_all = persist.tile([P, MB, S], dt.bfloat16, tag="yT")
            y_mb = [persist.tile([P, NKB, P], dt.bfloat16, tag=f"y_mb{m}",
                                 name=f"y_mb{m}")
                    for m in range(MB)]

            tri_sb = small.tile([P, P], dt.bfloat16, tag="tri")
            nc.sync.dma_start(out=tri_sb, in_=tri_d)
            ident_sb = small.tile([P, P], dt.bfloat16, tag="ident")
            nc.sync.dma_start(out=ident_sb, in_=ident_d)
            ones_sb = small.tile([1, DL], dt.bfloat16, tag="ones")
            nc.vector.memset(ones_sb, 1.0)

            # ones columns of v (softmax denominator accumulators)
            nc.vector.memset(v_sb[:, :, :, HEAD_DIM:HEAD_DIM + 1], 1.0)

            # ================= Stage B: projections + RoPE =================
            with (
                tc.tile_pool(name="bweights", bufs=1) as bweights,
                tc.tile_pool(name="bstage", bufs=3) as bstage,
                tc.tile_pool(name="proj_ps", bufs=3, space="PSUM") as proj_ps,
                tc.tile_pool(name="rot_ps", bufs=2, space="PSUM") as rot_ps,
            ):
                xT_sb = bweights.tile([P, KC, S], dt.bfloat16, tag="xT")
                for kc in range(KC):
                    nc.sync.dma_start(
                        out=xT_sb[:, kc, :], in_=xT_d[kc * P:(kc + 1) * P, :])
                w_sbs = {}
                b_sbs = {}
                for nm, wd in (("q", wq_d), ("k", wk_d), ("v", wv_d)):
                    w_sbs[nm] = bweights.tile([P, KC, DL], dt.bfloat16, tag=f"w{nm}", name=f"w{nm}")
                    for kc in range(KC):
                        nc.sync.dma_start(
                            out=w_sbs[nm][:, kc, :],
                            in_=wd[kc * P:(kc + 1) * P, :])
                for nm, bd in (("q", bq_d), ("k", bk_d)):
                    b_sbs[nm] = small.tile([P, MB], dt.float32, tag=f"b{nm}", name=f"b{nm}")
                    nc.sync.dma_start(out=b_sbs[nm], in_=bd)
                b_sbs["v"] = small.tile([1, DL], dt.bfloat16, tag="bv", name="bv")
                nc.sync.dma_start(out=b_sbs["v"], in_=bv_d)
                cos_sb = bweights.tile([P, S], dt.bfloat16, tag="cos")
                nc.sync.dma_start(out=cos_sb, in_=cos_d)
                sin_sb = bweights.tile([P, S], dt.bfloat16, tag="sin")
                nc.sync.dma_start(out=sin_sb, in_=sin_d)
                perm_sb = small.tile([P, P], dt.bfloat16, tag="perm")
                nc.sync.dma_start(out=perm_sb, in_=perm_d)

                NT = S // DL  # 4 tiles of 512 along sequence
                for nm in ("q", "k"):
                    w_sb, b_sb = w_sbs[nm], b_sbs[nm]
                    for m in range(MB):
                        for t in range(NT):
                            ps = proj_ps.tile([P, DL], dt.float32, tag="proj")
                            for kc in range(KC):
                                _tag(nc.tensor.matmul(
                                    ps, lhsT=w_sb[:, kc, m * P:(m + 1) * P],
                                    rhs=xT_sb[:, kc, t * DL:(t + 1) * DL],
                                    start=(kc == 0), stop=(kc == KC - 1)), "proj_qk")
                            raw = bstage.tile([P, DL], dt.bfloat16, tag="raw")
                            nc.vector.tensor_scalar(
                                raw, ps, b_sb[:, m:m + 1], None,
                                mybir.AluOpType.add)
                            rps = rot_ps.tile([P, DL], dt.float32, tag="rot")
                            _tag(nc.tensor.matmul(rps, lhsT=perm_sb, rhs=raw,
                                             start=True, stop=True), "rot")
                            rot = bstage.tile([P, DL], dt.bfloat16, tag="rot_sb")
                            nc.vector.tensor_copy(out=rot, in_=rps)
                            t1 = bstage.tile([P, DL], dt.bfloat16, tag="t1")
                            nc.gpsimd.tensor_mul(
                                t1, raw, cos_sb[:, t * DL:(t + 1) * DL])
                            t2 = bstage.tile([P, DL], dt.bfloat16, tag="t2")
                            nc.gpsimd.tensor_mul(
                                t2, rot, sin_sb[:, t * DL:(t + 1) * DL])
                            if nm == "q":
                                nc.vector.tensor_add(
                                    qTf[:, m, t * DL:(t + 1) * DL], t1, t2)
                            else:
                                for hh in range(2):
                                    po = hh * HEAD_DIM
                                    nc.vector.tensor_add(
                                        kTf[po:po + HEAD_DIM, 2 * m + hh,
                                            t * DL:(t + 1) * DL],
                                        t1[po:po + HEAD_DIM, :],
                                        t2[po:po + HEAD_DIM, :])

                # v projection: natural [seq, dims] layout + ones cols
                for kb in range(NKB):
                    ps = proj_ps.tile([P, DL], dt.float32, tag="proj")
                    for kc in range(KC):
                        _tag(nc.tensor.matmul(
                            ps, lhsT=xT_sb[:, kc, kb * P:(kb + 1) * P],
                            rhs=w_sbs["v"][:, kc, :],
                            start=(kc == 0), stop=False), "proj_v")
                    _tag(nc.tensor.matmul(
                        ps, lhsT=ones_sb[:, :P], rhs=b_sbs["v"],
                        start=False, stop=True), "bias_v")
                    nc.vector.tensor_copy(
                        out=v_sb[:, kb, :, 0:HEAD_DIM],
                        in_=ps.rearrange("p (h d) -> p h d", h=HPC))

            # ================= Stage C: attention per head =================
            with (
                tc.tile_pool(name="att_pool", bufs=2) as att_pool,
                tc.tile_pool(name="rtile", bufs=4) as rtile,
                tc.tile_pool(name="qk_ps", bufs=2, space="PSUM") as qk_ps,
                tc.tile_pool(name="y_ps", bufs=1, space="PSUM") as y_ps_pool,
            ):
                for h in range(HPC):
                    m, po = h // 2, (h % 2) * HEAD_DIM
                    qh = qTf[:, m, :]
                    kh = kTf[:, h, :]
                    att = att_pool.tile([P, ATT_TOT], dt.bfloat16, tag="att")

                    # C1: scores^T = k_ck^T q (per 128-key block), exp, mask
                    for ck in range(NKB):
                        w = S - ck * P
                        base = ck * P
                        off = ATT_OFF[ck]
                        for s0 in range(0, w, QK_PSUM_W):
                            sw = min(QK_PSUM_W, w - s0)
                            ps = qk_ps.tile([P, QK_PSUM_W], dt.float32, tag="qk")
                            for u0 in range(0, sw, DL):
                                uw = min(DL, sw - u0)
                                _tag(nc.tensor.matmul(
                                    ps[:, u0:u0 + uw],
                                    lhsT=kh[:, ck * P:(ck + 1) * P],
                                    rhs=qh[:, base + s0 + u0:base + s0 + u0 + uw],
                                    start=True, stop=True), "qk")
                            nc.scalar.activation(
                                out=att[:, off + s0:off + s0 + sw],
                                in_=ps[:, 0:sw], func=FExp, scale=SCALE)
                        # causal mask on the diagonal 128x128 block
                        nc.gpsimd.tensor_mul(
                            att[:, off:off + P], att[:, off:off + P], tri_sb)

                    # C2: y[qb] = sum_ck att[ck,qb-block]^T @ v_aug[ck]
                    for qb in range(NKB):
                        yp = y_ps_pool.tile([P, 2, HEAD_DIM + 1], dt.float32,
                                            tag="y")
                        ys = yp[:, qb % 2, :]
                        for ck in range(qb + 1):
                            a0 = ATT_OFF[ck] + (qb - ck) * P
                            _tag(nc.tensor.matmul(
                                ys, lhsT=att[:, a0:a0 + P],
                                rhs=v_sb[:, ck, h, :],
                                start=(ck == 0), stop=(ck == qb)), "av")
                        r = rtile.tile([P, 1], dt.float32, tag="r")
                        nc.vector.reciprocal(r, ys[:, HEAD_DIM:HEAD_DIM + 1])
                        nc.vector.tensor_scalar(
                            y_mb[m][:, qb, po:po + HEAD_DIM],
                            ys[:, 0:HEAD_DIM], r, None,
                            mybir.AluOpType.mult)

            # ============ Stage D: transpose y, out-projection ============
            with (
                tc.tile_pool(name="osb", bufs=3) as osb,
                tc.tile_pool(name="tp_ps", bufs=2, space="PSUM") as tp_ps,
                tc.tile_pool(name="o_ps", bufs=3, space="PSUM") as o_ps_pool,
            ):
                for m in range(MB):
                    for q4 in range(NKB // 4):
                        tp = tp_ps.tile([P, 4 * P], dt.bfloat16, tag="tp")
                        for j in range(4):
                            _tag(nc.tensor.transpose(
                                tp[:, j * P:(j + 1) * P],
                                y_mb[m][:, q4 * 4 + j, :], ident_sb), "ytrans")
                        nc.vector.tensor_copy(
                            out=yT_all[:, m, q4 * 4 * P:(q4 + 1) * 4 * P], in_=tp)

                for qb in range(NKB):
                    for t in range(2):
                        ps = o_ps_pool.tile([P, DL], dt.float32, tag="o")
                        for m in range(MB):
                            _tag(nc.tensor.matmul(
                                ps, lhsT=yT_all[:, m, qb * P:(qb + 1) * P],
                                rhs=wo_sb[:, m, t * DL:(t + 1) * DL],
                                start=(m == 0), stop=(m == MB - 1)), "oproj")
                        ob = osb.tile([P, DL], dt.float32, tag="ob")
                        nc.scalar.activation(out=ob, in_=ps, func=FCopy)
                        nc.sync.dma_start(
                            out=o_d[qb * P:(qb + 1) * P, t * DL:(t + 1) * DL],
                            in_=ob)

    nc.compile()
    return nc


def _host_tables():
    pos = np.arange(S, dtype=np.float32)
    freq = np.arange(0, HEAD_DIM, 2, dtype=np.float32) / HEAD_DIM
    inv_freq = 1.0 / (10000.0 ** freq)
    angles = np.outer(pos, inv_freq)                    # [S, 32]
    angles = np.repeat(angles, 2, axis=-1)              # [S, 64]
    cosT = np.tile(np.cos(angles).T, (2, 1))            # [128, S]
    sinT = np.tile(np.sin(angles).T, (2, 1))
    # rot(q)[2i] = -q[2i+1]; rot(q)[2i+1] = q[2i]; permT[j, p] = coeff of q[j]
    p64 = np.zeros((HEAD_DIM, HEAD_DIM), np.float32)
    for i in range(HEAD_DIM // 2):
        p64[2 * i + 1, 2 * i] = -1.0
        p64[2 * i, 2 * i + 1] = 1.0
    permT = np.zeros((P, P), np.float32)
    permT[:HEAD_DIM, :HEAD_DIM] = p64
    permT[HEAD_DIM:, HEAD_DIM:] = p64
    tri = np.triu(np.ones((P, P), np.float32))          # keep k<=q in [k,q]
    ident = np.eye(P, dtype=np.float32)
    return (cosT.astype(BF16), sinT.astype(BF16), permT.astype(BF16),
            tri.astype(BF16), ident.astype(BF16))


def kernel(x, Wq, bq, Wk, bk, Wv, bv, Wo, bo):
    from concourse.bass_utils import run_bass_kernel_spmd

    x = np.asarray(x, np.float32)
    Wq, Wk, Wv, Wo = (np.asarray(a, np.float32) for a in (Wq, Wk, Wv, Wo))
    bq, bk, bv, bo = (np.asarray(a, np.float32) for a in (bq, bk, bv, bo))

    if "nc" not in _CACHE:
        _CACHE["nc"] = _build_bass()
    nc = _CACHE["nc"]

    cosT, sinT, permT, tri, ident = _host_tables()
    consts = {"cosT": cosT, "sinT": sinT, "permT": permT, "tri": tri,
              "ident": ident}

    xTs = [np.ascontiguousarray(x[b].T).astype(BF16) for b in range(B)]
    in_maps = []
    for c in range(N_CORES):
        b, g = c // HG, c % HG
        sl = slice(g * DL, (g + 1) * DL)
        in_maps.append({
            "xT": xTs[b],
            "wq": np.ascontiguousarray(Wq[:, sl]).astype(BF16),
            "wk": np.ascontiguousarray(Wk[:, sl]).astype(BF16),
            "wv": np.ascontiguousarray(Wv[:, sl]).astype(BF16),
            "wo": np.ascontiguousarray(Wo[sl, :]).astype(BF16),
            "bqT": np.ascontiguousarray(bq[sl].reshape(MB, P).T).astype(np.float32),
            "bkT": np.ascontiguousarray(bk[sl].reshape(MB, P).T).astype(np.float32),
            "bv": bv[sl].reshape(1, DL).astype(BF16),
            **consts,
        })

    res = run_bass_kernel_spmd(nc, in_maps, core_ids=list(range(N_CORES)))
    _CACHE["last_result"] = res
    out = np.empty((B, S, D_MODEL), np.float32)
    for b in range(B):
        out[b] = res.results[HG * b]["o"] + res.results[HG * b + 1]["o"]
    out += bo.astype(np.float32)
    return out
